# revision 19
# baseline (speedup 1.0000x reference)
"""Bass/Trainium2 kernel for NF4-dequant (QLoRA-style) SwiGLU MLP.

Computation (matches the bitsandbytes-NF4 reference):
    dq_i = nf4_quant_dequant(w_i)   (per-64-block absmax scaling)
    out  = dq3-proj( silu(x @ dq1^T) * (x @ dq2^T) )

Sharding: tensor-parallel over the ffn dim H=11008 across 8 cores.
H is split in 64-aligned shards of width [1408 x4, 1344 x4]; the 1344
shards are zero-padded to 1408 so every core runs the same program.
Each core computes a full [T, D] partial of the down-projection; the
host sums the 8 partials (the TP all-reduce).

On-device pipeline per core:
  phase 1: stream natural-layout weight tiles, per-64-block absmax ->
           reciprocal -> normalize -> 15-boundary bucketize (fused
           compare*delta tensor_scalar ops + int16 add chain) ->
           rescale -> PE-transpose -> DRAM scratch.
  phase 2: gate/up matmuls (PSUM f32 accum over D), SiLU on ACT,
           h = silu(gate)*up on GPSIMD, h tiles spilled to DRAM.
  phase 3: down-proj partial matmuls, PSUM evict on DVE, DMA out.

Emission interleaves tb0's gate/up with the w1/w2 dequant and w3's
dequant with the later token blocks so DVE and PE overlap.

Env knobs (compile-time): KERNEL_MM=bf16|f32r, KERNEL_CMP_INT16=0|1.
"""

import os
import sys

import numpy as np

if not os.path.isdir(os.path.join(os.path.dirname(os.path.abspath(__file__)), "concourse")):
    for _p in ("/opt/trn_rl_repo",):
        if os.path.isdir(_p) and _p not in sys.path:
            sys.path.insert(0, _p)

import concourse.bass as bass
import concourse.mybir as mybir
import concourse.tile as tile
from concourse import bacc
from concourse.bass_utils import run_bass_kernel_spmd
from concourse.masks import make_identity

F32 = mybir.dt.float32
F32R = mybir.dt.float32r
BF16 = mybir.dt.bfloat16
I16 = mybir.dt.int16
OP = mybir.AluOpType

NF4_CODE = np.array(
    [
        -1.0, -0.6961928009986877, -0.5250730514526367, -0.39491748809814453,
        -0.28444138169288635, -0.18477343022823334, -0.09105003625154495, 0.0,
        0.07958029955625534, 0.16093020141124725, 0.24611230194568634,
        0.33791524171829224, 0.44070982933044434, 0.5626170039176941,
        0.7229568362236023, 1.0,
    ],
    dtype=np.float32,
)
NF4_BOUNDS = ((NF4_CODE[:-1] + NF4_CODE[1:]) * np.float32(0.5)).astype(np.float32)
NF4_DELTAS = (NF4_CODE[1:] - NF4_CODE[:-1]).astype(np.float32)
VSCALE = 16384.0  # codes reconstructed as int16 / 2^14
NF4_IDELTAS = [int(x) for x in np.round(NF4_DELTAS * VSCALE)]
CSCALE = 32760.0  # int16 compare-domain scale (saturation-safe)
NF4_IBOUNDS = [int(np.floor(float(b) * CSCALE)) for b in NF4_BOUNDS]

BLK = 64

D = 4096
T_FULL = 4096
H_FULL = 11008
N_CORES = 8
HP = 1408
SHARD_W = [1408, 1408, 1408, 1408, 1344, 1344, 1344, 1344]
SHARD_START = [0, 1408, 2816, 4224, 5632, 6976, 8320, 9664]

NSUB = 512
KT = D // 128  # 32
HT = HP // 128  # 11

MM_MODE = os.environ.get("KERNEL_MM", "bf16")  # bf16 | f32r
CMP_INT16 = os.environ.get("KERNEL_CMP_INT16", "0") == "1"

if MM_MODE == "bf16":
    WDT = BF16  # storage dtype for dq scratch / x / h
    MMDT = None  # no bitcast needed
    T_BLK = 1024
else:
    WDT = F32
    MMDT = F32R
    T_BLK = 512
NTB = T_FULL // T_BLK


def _mm(ap):
    return ap.bitcast(MMDT) if MMDT is not None else ap


class P:
    pass


def _emit_dequant_tile(nc, p, w_ap, row0, col0, cw, store_fn, identity):
    """Dequantize a [128, cw] natural tile; write transposed blocks."""
    nblk = cw // BLK

    wt = p.pw.tile([128, cw], F32, tag="wt", name="wt")
    nc.sync.dma_start(wt[:], w_ap[row0 : row0 + 128, col0 : col0 + cw])
    w3v = wt[:].rearrange("p (b i) -> p b i", i=BLK)

    amax = p.pa.tile([128, nblk], F32, tag="amax", name="amax")
    nc.vector.tensor_reduce(
        amax[:], w3v, axis=mybir.AxisListType.X, op=OP.max, apply_absolute_value=True
    )
    aclamp = p.pa.tile([128, nblk], F32, tag="aclamp", name="aclamp")
    nc.vector.tensor_scalar_max(aclamp[:], amax[:], 1e-35)
    recip = p.pa.tile([128, nblk], F32, tag="recip", name="recip")
    nc.vector.reciprocal(recip[:], aclamp[:])
    av = p.pa.tile([128, nblk], F32, tag="av", name="av")
    nc.vector.tensor_scalar_mul(av[:], amax[:], 1.0 / VSCALE)
    av_b = av[:].unsqueeze(2).broadcast_to([128, nblk, BLK])

    if CMP_INT16:
        rs = p.pa.tile([128, nblk], F32, tag="rs", name="rs")
        nc.vector.tensor_scalar_mul(rs[:], recip[:], CSCALE)
        r_b = rs[:].unsqueeze(2).broadcast_to([128, nblk, BLK])
        vn = p.pvn.tile([128, cw], I16, tag="vn", name="vn")
        bounds = NF4_IBOUNDS
    else:
        r_b = recip[:].unsqueeze(2).broadcast_to([128, nblk, BLK])
        vn = p.pvn.tile([128, cw], F32, tag="vn", name="vn")
        bounds = [float(b) for b in NF4_BOUNDS]
    vn3 = vn[:].rearrange("p (b i) -> p b i", i=BLK)
    nc.vector.tensor_tensor(vn3, w3v, r_b, OP.mult)

    def prod(j, out_ap):
        nc.vector.tensor_scalar(
            out_ap, vn[:], bounds[j], NF4_IDELTAS[j], OP.is_gt, OP.mult
        )

    acc = p.pchain.tile([128, cw], I16, tag="acc", name="acc")
    prod(0, acc[:])
    for j in range(1, 15):
        tmp = p.pprod.tile([128, cw], I16, tag="prod", name="tmp")
        prod(j, tmp[:])
        nc.vector.tensor_tensor(acc[:], acc[:], tmp[:], OP.add)

    dq = p.pdq.tile([128, cw], WDT, tag="dq", name="dq")
    dq3 = dq[:].rearrange("p (b i) -> p b i", i=BLK)
    acc3 = acc[:].rearrange("p (b i) -> p b i", i=BLK)
    # dq = (acc - 2^14) * (absmax / 2^14)
    nc.vector.scalar_tensor_tensor(dq3, acc3, -int(VSCALE), av_b, OP.add, OP.mult)

    for jb in range(cw // 128):
        ps = p.pps.tile([128, 128], WDT, tag="ps", name="tps")
        nc.tensor.transpose(ps[:], dq[:, jb * 128 : (jb + 1) * 128], identity[:])
        qt = p.pqt.tile([128, 128], WDT, tag="qt", name="qt")
        nc.scalar.copy(qt[:], ps[:])
        nc.gpsimd.dma_start(store_fn((col0 + jb * 128) // 128), qt[:])


def _build_program():
    nc = bacc.Bacc("TRN2", target_bir_lowering=False, debug=False, num_devices=N_CORES)

    xT = nc.dram_tensor("xT", [D, T_FULL], F32, kind="ExternalInput").ap()
    w1s = nc.dram_tensor("w1s", [HP, D], F32, kind="ExternalInput").ap()
    w2s = nc.dram_tensor("w2s", [HP, D], F32, kind="ExternalInput").ap()
    w3s = nc.dram_tensor("w3s", [D, HP], F32, kind="ExternalInput").ap()
    out = nc.dram_tensor("out", [T_FULL, D], F32, kind="ExternalOutput").ap()

    from contextlib import ExitStack

    with tile.TileContext(nc) as tc, ExitStack() as ctx:
        p = P()
        dram = ctx.enter_context(tc.tile_pool(name="dram", bufs=1, space="DRAM"))
        s1 = dram.tile([HT, 128, KT, 128], WDT)
        s2 = dram.tile([HT, 128, KT, 128], WDT)
        s3 = dram.tile([HT, 128, D], WDT)
        hTd = dram.tile([NTB, HT, 128, T_BLK], WDT)

        const = ctx.enter_context(tc.tile_pool(name="const", bufs=1))
        identity = const.tile([128, 128], WDT)
        make_identity(nc, identity[:])

        if MM_MODE == "bf16":
            pool_spec = [
                ("pw", 2), ("pa", 2), ("pvn", 2), ("pprod", 6), ("pchain", 4),
                ("pdq", 2), ("pqt", 4), ("px", 2), ("pxb", KT), ("pl", 2),
                ("pht", 3), ("psl", 4), ("pu", 4), ("phl", HT + 1), ("pr3", 3),
                ("pob", 4),
            ]
        else:
            pool_spec = [
                ("pw", 2), ("pa", 2), ("pvn", 2), ("pprod", 4), ("pchain", 4),
                ("pdq", 2), ("pqt", 2), ("px", KT), ("pl", 2),
                ("pht", 3), ("psl", 2), ("pu", 2), ("phl", HT + 1), ("pr3", 3),
                ("pob", 2),
            ]
        for nm, bufs in pool_spec:
            setattr(p, nm, ctx.enter_context(tc.tile_pool(name=nm, bufs=bufs)))
        p.pps = ctx.enter_context(tc.tile_pool(name="pps", bufs=8, space="PSUM"))

        def dq_tile_w12(which, s, i):
            w_ap = w1s if which == 1 else w2s
            for ch in range(0, D, 1024):
                _emit_dequant_tile(
                    nc, p, w_ap, i * 128, ch, 1024,
                    lambda kt, i=i: s[i, :, kt, :], identity,
                )

        def load_x(tb):
            xk = []
            for k in range(KT):
                xf = p.px.tile([128, T_BLK], F32, tag="xf", name="xf")
                nc.sync.dma_start(
                    xf[:], xT[k * 128 : (k + 1) * 128, tb * T_BLK : (tb + 1) * T_BLK]
                )
                if MM_MODE == "bf16":
                    xb = p.pxb.tile([128, T_BLK], BF16, tag="xb", name="xb")
                    nc.scalar.copy(xb[:], xf[:])
                    xk.append(xb)
                else:
                    xk.append(xf)
            return xk

        def phase2_htile(tb, h, xk):
            l1 = p.pl.tile([128, KT * 128], WDT, tag="l1", name="l1")
            nc.sync.dma_start(l1[:], s1[h, :, :, :].rearrange("p k i -> p (k i)"))
            l2 = p.pl.tile([128, KT * 128], WDT, tag="l2", name="l2")
            nc.sync.dma_start(l2[:], s2[h, :, :, :].rearrange("p k i -> p (k i)"))
            ht = p.pht.tile([128, T_BLK], WDT, tag="ht", name="ht")
            for c in range(T_BLK // NSUB):
                pg = p.pps.tile([128, NSUB], F32, tag="ps", name="pg")
                pu = p.pps.tile([128, NSUB], F32, tag="ps", name="pu")
                for k in range(KT):
                    nc.tensor.matmul(
                        pg[:],
                        _mm(l1[:, k * 128 : (k + 1) * 128]),
                        _mm(xk[k][:, c * NSUB : (c + 1) * NSUB]),
                        start=(k == 0),
                        stop=(k == KT - 1),
                    )
                for k in range(KT):
                    nc.tensor.matmul(
                        pu[:],
                        _mm(l2[:, k * 128 : (k + 1) * 128]),
                        _mm(xk[k][:, c * NSUB : (c + 1) * NSUB]),
                        start=(k == 0),
                        stop=(k == KT - 1),
                    )
                sl = p.psl.tile([128, NSUB], WDT, tag="sl", name="sl")
                nc.scalar.activation(sl[:], pg[:], mybir.ActivationFunctionType.Silu)
                ue = p.pu.tile([128, NSUB], WDT, tag="ue", name="ue")
                nc.scalar.copy(ue[:], pu[:])
                nc.gpsimd.tensor_tensor(
                    ht[:, c * NSUB : (c + 1) * NSUB], sl[:], ue[:], OP.mult
                )
            nc.gpsimd.dma_start(hTd[tb, h, :, :], ht[:])

        def phase3(tb):
            strips = []
            for k in range(HT):
                hl = p.phl.tile([128, T_BLK], WDT, tag="hl", name="hl")
                nc.sync.dma_start(hl[:], hTd[tb, k, :, :])
                strips.append(hl)
            for dc in range(D // NSUB):
                po = [
                    p.pps.tile([128, NSUB], F32, tag="ps", name=f"po{tt}")
                    for tt in range(T_BLK // 128)
                ]
                for k in range(HT):
                    r3 = p.pr3.tile([128, NSUB], WDT, tag="r3", name="r3")
                    nc.sync.dma_start(r3[:], s3[k, :, dc * NSUB : (dc + 1) * NSUB])
                    for tt in range(T_BLK // 128):
                        nc.tensor.matmul(
                            po[tt][:],
                            _mm(strips[k][:, tt * 128 : (tt + 1) * 128]),
                            _mm(r3[:]),
                            start=(k == 0), stop=(k == HT - 1),
                        )
                for tt in range(T_BLK // 128):
                    ob = p.pob.tile([128, NSUB], F32, tag="ob", name="ob")
                    nc.vector.tensor_copy(ob[:], po[tt][:])
                    nc.gpsimd.dma_start(
                        out[
                            tb * T_BLK + tt * 128 : tb * T_BLK + (tt + 1) * 128,
                            dc * NSUB : (dc + 1) * NSUB,
                        ],
                        ob[:],
                    )

        w3_work = [
            (i, ch, cw) for i in range(KT) for (ch, cw) in [(0, 640), (640, 768)]
        ]
        w3_iter = iter(w3_work)

        def emit_w3(n):
            for _ in range(n):
                item = next(w3_iter, None)
                if item is None:
                    return
                i, ch, cw = item
                _emit_dequant_tile(
                    nc, p, w3s, i * 128, ch, cw,
                    lambda hb, i=i: s3[hb, :, i * 128 : (i + 1) * 128], identity,
                )

        xk0 = load_x(0)
        for i in range(HT):
            dq_tile_w12(1, s1, i)
            dq_tile_w12(2, s2, i)
            phase2_htile(0, i, xk0)
        n_slots = (NTB - 1) * HT
        per_slot = -(-len(w3_work) // n_slots) if n_slots else len(w3_work)
        for tb in range(1, NTB):
            xk = load_x(tb)
            for h in range(HT):
                emit_w3(per_slot)
                phase2_htile(tb, h, xk)
        emit_w3(len(w3_work))
        for tb in range(NTB):
            phase3(tb)

    nc.compile()
    return nc


_CACHED_NC = None
LAST_RESULTS = None


def _shard_inputs(x, w1, w2, w3):
    xT = np.ascontiguousarray(x.reshape(T_FULL, D).T, dtype=np.float32)
    in_maps = []
    for c in range(N_CORES):
        s, w = SHARD_START[c], SHARD_W[c]
        w1c = np.zeros((HP, D), dtype=np.float32)
        w1c[:w] = w1[s : s + w]
        w2c = np.zeros((HP, D), dtype=np.float32)
        w2c[:w] = w2[s : s + w]
        w3c = np.zeros((D, HP), dtype=np.float32)
        w3c[:, :w] = w3[:, s : s + w]
        in_maps.append({"xT": xT, "w1s": w1c, "w2s": w2c, "w3s": w3c})
    return in_maps


def kernel(x, w1, w2, w3):
    global _CACHED_NC, LAST_RESULTS
    assert x.shape == (2, 2048, D) and w1.shape == (H_FULL, D)
    if _CACHED_NC is None:
        _CACHED_NC = _build_program()
    in_maps = _shard_inputs(x, w1, w2, w3)
    res = run_bass_kernel_spmd(
        _CACHED_NC,
        in_maps,
        core_ids=list(range(N_CORES)),
        trace=os.environ.get("KERNEL_TRACE", "") == "1",
    )
    LAST_RESULTS = res
    acc = res.results[0]["out"].astype(np.float32).copy()
    for c in range(1, N_CORES):
        acc += res.results[c]["out"]
    return acc.reshape(2, 2048, D).astype(np.float32)
